# revision 6
# baseline (speedup 1.0000x reference)
"""Distributed causal attention head for TRN2 (8 NeuronCores), v3.

Problem: B=4, S=4096, D=1024, H=64 fp32.
  q,k,v = x @ W{q,k,v}; scores = q k^T / sqrt(H); causal softmax; out = P v.

Sharding (fully SPMD-uniform, one NEFF for all 8 cores):
  - 4 batches x 2 cores per batch (pair replica groups [[0,1],[2,3],[4,5],[6,7]]).
  - Within a pair the KEY dimension is split by interleaved 128-row chunks:
    core g owns global key chunks {2i+g}. Host pre-transposes each core's
    2048 input rows to x^T [1024, 2048] bf16 so all DMA loads are linear
    (2 KB per-partition runs). Weights/masks are host-relaid partition-major.
  - Q^T is pair-AllGathered in TWO pieces (local key-chunk halves) so the
    first attention blocks can start while the second AllGather is in flight;
    the natural-V projection runs inside the AllGather latency window.
  - Per-core partial (numerator | denominator) = [65, 4096] fp32 goes straight
    to DRAM; the HOST merges the pair (add), divides, and transposes. No
    ReduceScatter on device.

Compute layout:
  - k|q packed projection contracts on the partition dim, x^T tiles moving
    (N=1024); V is computed in natural [keys, 64] layout with x^T chunks
    stationary (no PE transposes anywhere in the kernel).
  - Scores transposed with 2x PE row tiling (64-contract): tile (0,0) does
    local chunks 0..t of q-block t, tile (64,0) chunks t+1..2t+1. kT/qT are
    duplicated into both SBUF partition halves (SBUF->SBUF DMA).
  - Gathered Q^T stays source-major [128, 2, 16, 128]; score matmuls use two
    N=256 strided-AP pieces so no interleave scatter-DMA is needed.
  - exp on the scalar engine over 3-bank PSUM score sets (N=1536, 24 uniform
    sets of 3 chunks), scale=1/8 fused, bf16 out. AV matmuls are emitted
    per-set right after each exp (event-driven) to avoid PE bursts.
  - V is augmented with a ones column so the AV matmul also produces the
    softmax denominator (row 64 of the [65, 512] accumulator).
"""

import sys

sys.path.insert(0, "/opt/trn_rl_repo")

import numpy as np
import ml_dtypes

B, S, D, H = 4, 4096, 1024, 64
RPC = S // 2            # rows (keys/queries) owned per core
QB = 512                # query block width
NQB = S // QB           # 8 query blocks
NKC = RPC // 128        # 16 local key chunks
HKC = NKC // 2          # chunks per AllGather half
BF16 = ml_dtypes.bfloat16
PAIRS = [[0, 1], [2, 3], [4, 5], [6, 7]]

_CACHE = {}


def _build():
    import concourse.bass as bass
    import concourse.mybir as mybir
    from concourse import bacc, tile
    from concourse.bass import ts

    f32 = mybir.dt.float32
    bf16 = mybir.dt.bfloat16
    Alu = mybir.AluOpType
    Act = mybir.ActivationFunctionType

    nc = bacc.Bacc(None, target_bir_lowering=False)

    x_ext = nc.declare_dram_parameter("x", [D, RPC], bf16, isOutput=False)
    wkq_ext = nc.declare_dram_parameter("wkq", [128, 8 * 128], bf16, isOutput=False)
    wv_ext = nc.declare_dram_parameter("wv", [128, 8 * H], bf16, isOutput=False)
    mask_ext = nc.declare_dram_parameter("mask", [128, 2 * QB], bf16, isOutput=False)
    out_ext = nc.declare_dram_parameter("out", [H + 1, S], f32, isOutput=True)

    with tile.TileContext(nc) as tc:
        with (
            tc.tile_pool(name="persist", bufs=1) as persist,
            tc.tile_pool(name="dram", bufs=1, space="DRAM") as dram,
        ):
            # --- persistent SBUF tensors ---
            xT = persist.tile([128, 8, RPC], bf16, tag="xT")
            wkq_sb = persist.tile([128, 8, 128], bf16, tag="wkq")
            wv_sb = persist.tile([128, 8, H], bf16, tag="wv")
            mask_sb = persist.tile([128, 2, QB], bf16, tag="mask")
            kT2 = persist.tile([128, NKC, 128], bf16, tag="kT2")
            qT2g = persist.tile([128, 2, NKC, 128], bf16, tag="qT2g")
            v_all = persist.tile([128, NKC, H + 2], bf16, tag="v_all")
            qtmp = persist.tile([128, RPC], bf16, tag="qtmp")
            zjunk = persist.tile([128, 8], f32, tag="zjunk")
            ejunk = persist.tile([128, 8], bf16, tag="ejunk")

            # preload the exp activation table set early (it costs ~2.7us)
            nc.vector.memset(zjunk[:], 0.0)
            nc.scalar.activation(ejunk[:], zjunk[:], Act.Exp)
            nc.vector.memset(v_all[:, :, H], 1.0)

            # x loads first (bulk), then the small contiguous weight/mask loads
            for h in range(2):
                for dc in range(8):
                    nc.sync.dma_start(
                        out=xT[:, dc, ts(h, RPC // 2)],
                        in_=x_ext[ts(dc, 128), ts(h, RPC // 2)],
                    )
            nc.sync.dma_start(out=wkq_sb[:], in_=wkq_ext[:])
            nc.sync.dma_start(out=wv_sb[:], in_=wv_ext[:])
            nc.sync.dma_start(out=mask_sb[:], in_=mask_ext[:])

            q_bounce = []
            q_gath = []
            for h in range(2):
                qb = dram.tile([64, RPC // 2], bf16, tag=f"q_bounce{h}")
                qg = dram.tile([2, 64, RPC // 2], bf16, tag=f"q_gath{h}")
                q_bounce.append(qb)
                q_gath.append(qg)

            # --- phase 1: k|q projections per column half + early AllGathers ---
            with (
                tc.tile_pool(name="pj", bufs=2, space="PSUM") as pj_pool,
                tc.tile_pool(name="pv", bufs=2, space="PSUM") as pv_pool,
            ):
                for h in range(2):
                    kq_ps = pj_pool.tile([128, RPC // 2], f32, tag="kq")
                    for pp in range(2):
                        for dc in range(8):
                            nc.tensor.matmul(
                                kq_ps[:, ts(pp, QB)],
                                lhsT=wkq_sb[:, dc, :],
                                rhs=xT[:, dc, h * (RPC // 2) + pp * QB : h * (RPC // 2) + (pp + 1) * QB],
                                start=(dc == 0),
                                stop=(dc == 7),
                            )
                    for kc in range(HKC):
                        nc.any.tensor_copy(
                            kT2[0:64, HKC * h + kc, :], kq_ps[0:64, ts(kc, 128)]
                        )
                    nc.any.tensor_copy(qtmp[64:128, ts(h, RPC // 2)], kq_ps[64:128, :])
                    nc.sync.dma_start(
                        out=q_bounce[h][:], in_=qtmp[64:128, ts(h, RPC // 2)]
                    )
                    nc.gpsimd.collective_compute(
                        "AllGather",
                        Alu.bypass,
                        replica_groups=PAIRS,
                        ins=[q_bounce[h].opt()],
                        outs=[q_gath[h].opt()],
                    )

                # --- natural-layout V inside the AllGather latency window ---
                for kc in range(NKC):
                    v_ps = pv_pool.tile([128, H], f32, tag="v")
                    for dc in range(8):
                        nc.tensor.matmul(
                            v_ps[:],
                            lhsT=xT[:, dc, ts(kc, 128)],
                            rhs=wv_sb[:, dc, :],
                            start=(dc == 0),
                            stop=(dc == 7),
                        )
                    nc.any.tensor_copy(v_all[:, kc, 0:H], v_ps[:])

            # kT high-half duplicate (SBUF->SBUF, partition shift)
            nc.sync.dma_start(out=kT2[64:128, :, :], in_=kT2[0:64, :, :])
            # gathered q into both partition halves, source-major layout
            for h in range(2):
                for src in range(2):
                    nc.sync.dma_start(
                        out=qT2g[0:64, src, ts(h, HKC), :], in_=q_gath[h][src]
                    )
                    nc.sync.dma_start(
                        out=qT2g[64:128, src, ts(h, HKC), :], in_=q_gath[h][src]
                    )

            # --- phase 2: attention ---
            with (
                tc.tile_pool(name="st", bufs=2, space="PSUM") as st_pool,
                tc.tile_pool(name="av", bufs=2, space="PSUM") as av_pool,
                tc.tile_pool(name="p", bufs=3) as p_pool,
                tc.tile_pool(name="o", bufs=3) as o_pool,
            ):
                gamma = 0
                cur_st = None
                cur_p = None
                pend_masks = []
                set_chunks = []
                av_tiles = {}

                for t in range(NQB):
                    E = 2 * (t + 1)
                    for s in range(t + 1):
                        for hh in (0, 1):
                            cid = s if hh == 0 else t + 1 + s
                            if cur_st is None:
                                cur_st = st_pool.tile([128, 3, QB], f32, tag="st")
                                cur_p = p_pool.tile([128, 3, QB], bf16, tag="p")
                            pos = gamma % 3
                            for pc in (0, 1):
                                nc.tensor.matmul(
                                    cur_st[:, pos, ts(pc, 256)],
                                    lhsT=kT2[64 * hh : 64 * hh + 64, cid, :],
                                    rhs=qT2g[64 * hh : 64 * hh + 64, :, 2 * t + pc, :],
                                    start=True,
                                    stop=True,
                                    tile_position=(64 * hh, 0),
                                )
                            set_chunks.append((t, cid, pos))
                            if cid >= E - 2:
                                pend_masks.append((pos, cid - (E - 2)))
                            gamma += 1
                            if gamma % 3 == 0:
                                nc.scalar.activation(
                                    cur_p[:], cur_st[:], Act.Exp, scale=0.125
                                )
                                for pp, j in pend_masks:
                                    nc.vector.tensor_tensor(
                                        cur_p[:, pp, :],
                                        cur_p[:, pp, :],
                                        mask_sb[:, j, :],
                                        Alu.mult,
                                    )
                                pend_masks = []
                                for tt, cc, pp in set_chunks:
                                    if cc == 0:
                                        av_new = av_pool.tile(
                                            [H + 1, QB], f32, tag="av"
                                        )
                                        av_tiles[tt] = av_new
                                    nc.tensor.matmul(
                                        av_tiles[tt][:],
                                        lhsT=v_all[:, cc, 0 : H + 1],
                                        rhs=cur_p[:, pp, :],
                                        start=(cc == 0),
                                        stop=(cc == 2 * tt + 1),
                                    )
                                    if cc == 2 * tt + 1:
                                        o = o_pool.tile([H + 1, QB], f32, tag="o")
                                        nc.vector.tensor_copy(o[:], av_tiles[tt][:])
                                        nc.sync.dma_start(
                                            out=out_ext[:, ts(tt, QB)], in_=o[:]
                                        )
                                        del av_tiles[tt]
                                set_chunks = []
                                cur_st = None
                                cur_p = None

    nc.finalize()
    return nc


def _make_masks(g: int) -> np.ndarray:
    # mask[j][kk, qq] = 1 if query (512t + qq) >= key 128*(4t + 2j + g) + kk
    m = np.zeros((2, 128, QB), dtype=np.float32)
    for j in range(2):
        dk = 128 * (2 * j + g) + np.arange(128)[:, None]
        dq = np.arange(QB)[None, :]
        m[j] = (dq >= dk).astype(np.float32)
    return m.astype(BF16)


def _shard_inputs(input, Wq, Wk, Wv):
    x = np.asarray(input)
    wkq = np.concatenate([Wk, Wq], axis=1).astype(np.float32)  # [D, 128]
    # partition-major relayout: wkq_h[p, dc*128+j] = wkq[dc*128+p, j]
    wkq_h = np.ascontiguousarray(
        wkq.reshape(8, 128, 128).transpose(1, 0, 2).reshape(128, 8 * 128)
    ).astype(BF16)
    wv_h = np.ascontiguousarray(
        np.asarray(Wv, dtype=np.float32).reshape(8, 128, H).transpose(1, 0, 2).reshape(128, 8 * H)
    ).astype(BF16)
    masks = []
    for g in range(2):
        m = _make_masks(g)  # [2, 128, QB]
        masks.append(np.ascontiguousarray(m.transpose(1, 0, 2).reshape(128, 2 * QB)))
    in_maps = []
    for c in range(8):
        b, g = c // 2, c % 2
        xs = x[b].reshape(S // 128, 128, D)[g::2].reshape(RPC, D)
        xT = np.ascontiguousarray(xs.T).astype(BF16)
        in_maps.append({"x": xT, "wkq": wkq_h, "wv": wv_h, "mask": masks[g]})
    return in_maps


def _unshard(results):
    out = np.empty((B, S, H), dtype=np.float32)
    for b in range(B):
        merged = results[2 * b]["out"] + results[2 * b + 1]["out"]
        out[b] = (merged[:H] / merged[H : H + 1]).T
    return out


def _run(inputs, trace=False):
    from concourse.bass_utils import run_bass_kernel_spmd

    if "nc" not in _CACHE:
        _CACHE["nc"] = _build()
    nc = _CACHE["nc"]
    in_maps = _shard_inputs(**inputs)
    res = run_bass_kernel_spmd(nc, in_maps, core_ids=list(range(8)), trace=trace)
    out = _unshard(res.results)
    return out, res


def kernel(**inputs) -> np.ndarray:
    out, _ = _run(inputs, trace=False)
    return out
